# revision 36
# baseline (speedup 1.0000x reference)
"""Trainium2 Bass kernel for nn_KC_Avg_Embedding (multi-hot averaged embedding).

Computes, for multi-hot indicator vectors x[b,s,:] over a vocabulary of 1024:
    out[b,s,:] = (x[b,s,:] @ E) / max(sum(x[b,s,:]), 1)

Strategy (data-parallel over 8 NeuronCores, batch-sharded; memory-regime):
  - Each core gets rows = (B/8)*S = 3200 rows = 25 row-tiles of 128. Host
    uploads x^T pre-swizzled to [vocab-part, chunk, row] fp8 (x is 0/1 so
    fp8 is exact); E is fp16 with a ones column appended so the averaging
    denominator falls out of the same matmuls (psum col 128 = row count).
  - Per 128-row tile: 8 accumulating matmuls (lhsT = x^T chunk fp8
    stationary, rhs = E_aug fp16 moving, N=129) -> PSUM [128,129] f32.
  - Per-core HBM bandwidth is ~358 GB/s TOTAL across all DMA queues while
    the PE eats x at ~277 GB/s, so delivery must be in consumption order:
    the two HWDGE rings alternate fine groups early / coarse late, sized so
    each group completes just before the PE needs its first tile. E and
    tile0 ride ONE byte-blob DMA (bitcast views) so the matmul gate is a
    single completion; tile1 rides ring B concurrently.
  - Epilogue: tiles 0-17 stage PSUM->SBUF f16 via scalar ACT (its one
    sync-wait = the PE tick; the PSUM-recycling matmul's one wait = the
    ACT tick — this walrus build caps every instruction at ONE sync-wait),
    then DVE reciprocal of the count column + scale-mul into out_sb.
    Tiles 18-24 own the last PSUM rotation (never recycled), so they skip
    the ACT and run recip+mul straight from PSUM, shortening the tail.
  - Outputs: p-major y [128, 25, 128] f16 (host untransposes). Tiles 0-13
    drain on SWDGE mid-run (a 64B priming DMA at start absorbs the
    software DGE's ~3.5us cold boot; NB a 4B-descriptor DMA wedges the
    device); (14-22) rides ring A, the final 2 tiles ride ring B. All
    out_sb writes are DVE muls -> every output DMA is a single-sem join.
    The two HWDGE output DMAs are the 11th/12th HWDGE transfers: they
    reuse DMAHW lanes, and the framework's lane-serialization wait is
    hoisted onto carrier nops (same engine, just before) to respect the
    1-sync-wait cap.
  - ~30 dummy matmuls on garbage-zeroed tiles warm the PE HAM clock gate
    (cold clock is 1.2 vs 2.4 GHz; the ramp takes ~3.6us of PE activity),
    timed to drain right as the blob lands.
  - HBM per core: 3.28 MB (x) + 0.26 MB (E) + 0.82 MB (y) ~= 4.4 MB.
"""

import sys
from contextlib import ExitStack

import numpy as np
import ml_dtypes

for _p in ("/opt/trn_rl_repo",):
    if _p not in sys.path:
        sys.path.insert(0, _p)

import concourse.bass as bass
import concourse.mybir as mybir
import concourse.tile as tile

from concourse.vector_clock import ScopedClock


class _SplitDrainTC(tile.TileContext):
    """TileContext whose kernel-tail drain splits its semaphore waits across
    single-wait carrier nops — this walrus build enforces a small
    per-instruction sync-wait limit that the stock all-lane drain exceeds.

    If ``precleared`` covers every sem the context allocated, the tail
    semaphore clear + its barrier are skipped: the clear instead runs at
    the START of the next execution (hidden between the construction
    barrier and the context entry barrier), saving ~0.4us of teardown."""

    precleared = frozenset()

    def _drain_and_barrier(self, tick_clock, wait_clock):
        drain_inst = self.nc.sync.drain()
        wait_clock.add_sem_waits(
            drain_inst.ins, ScopedClock({None: tick_clock.global_clock})
        )
        si = drain_inst.ins.sync_info
        if si is not None and si.on_wait is not None:
            # Input-DMA lane waits (DMAHW at final value 16 — one input per
            # lane) are dominated by the PE tick: every input byte was
            # consumed by a matmul whose LDW waited that lane. Dropping
            # them removes ~8 serialized carrier nops from the teardown.
            # Output lanes are reused lanes (final value 32) and DMASW
            # lanes (SWDGE output) — kept.
            kept = [w for w in si.on_wait
                    if not ((w.ant_name or "").startswith("DMAHW")
                            and (w.wait_value or 0) <= 16)]
            if kept:
                del si.on_wait[:]
                si.on_wait.extend(kept)
        if si is not None and si.on_wait is not None and len(si.on_wait) > 1:
            waits = list(si.on_wait)
            del si.on_wait[1:]
            for w in waits[1:]:
                nop = self.nc.sync.nop(nofuse=True, hint="drain_wait_split")
                nsi = nop.ins.sync_info
                if nsi is None:
                    nop.ins.sync_info = mybir.SyncInfo(on_update=[], on_wait=[w])
                else:
                    nsi.on_wait.append(w)
        assert self.sems is not None
        popped = self.nc._tile_sem_poison_stack.pop()
        assert popped is self._sem_poison
        used = list(self.sems.allocated().values())
        if all(s.num in self.precleared for s in used):
            # no barrier needed: the sync drain holds the data-integrity
            # waits, engines re-sync at the NEFF postamble round, and the
            # sem clear runs at the next execution's startup
            pass
        else:
            self.nc.all_engine_barrier()
            self.nc.clear_and_free_semaphores(used)
            self.nc.all_engine_barrier()


B, S, V, D = 128, 200, 1024, 128
NCORES = 8
P = 128
PER_CORE_B = B // NCORES          # 16
ROWS = PER_CORE_B * S             # 3200 rows per core
NCH = V // P                      # 8 vocab chunks
NE = D + 1                        # 128 emb cols + 1 count col
NT = ROWS // P                    # 25 row tiles
EBYTES = NCH * NE * 2             # 2064 E bytes per partition
XTBYTES = NCH * P                 # 1024 x bytes per partition per tile
WARMUP_MM = 28                    # dummy matmuls to warm the PE HAM clock gate
                                  # (drains ~10.3-10.8us: covers the ramp but
                                  # never outlasts the blob gate when the
                                  # preamble jitters late)
DIRECT_FROM = NT - 7              # tiles 18+: PSUM never recycled -> direct DVE

# input groups in consumption order, alternating rings: ('a'|'b', lo, hi)
# tile 0 rides the blob on ring A; tile 1 rides ring B concurrently.
# Fine groups up front (each must land ~1.5us before the PE needs it —
# DMA completion sems trail the wire by up to 1.4us), coarse at the back.
IN_GROUPS = [
    ("b", 1, 2), ("a", 2, 3), ("a", 3, 4), ("b", 4, 6), ("a", 6, 8),
    ("b", 8, 11), ("a", 11, 14), ("b", 14, 18), ("a", 18, 25),
]
# output chunks [lo,hi): SWDGE early bulk, then both late chunks on ring
# A — outF's descriptors are generated (on sync, in order behind out1's)
# while the ring is still draining out1, so the ring picks them up with
# no fresh pickup/boot idle, unlike a cold handoff to ring B.
OUT_CHUNKS = [("s", 0, 14), ("a", 14, 20), ("a", 20, 23), ("a", 23, 25)]

FP8 = mybir.dt.float8e4
F16 = mybir.dt.float16
F32 = mybir.dt.float32
U8 = mybir.dt.uint8
NP_FP8 = ml_dtypes.float8_e4m3
FP8_ONE = 0x38                    # bit pattern of 1.0 in fp8e4m3


def build_kernel():
    nc = bass.Bass()
    # [E_aug f16 | x^T tile0 fp8] as one byte tensor: one DMA completion
    # gates the first matmul on both.
    blob = nc.declare_dram_parameter("blob", [P, EBYTES + XTBYTES], U8,
                                     isOutput=False)
    gparams = [
        nc.declare_dram_parameter(f"g{i}", [P, NCH, (hi - lo) * P], FP8,
                                  isOutput=False)
        for i, (ch, lo, hi) in enumerate(IN_GROUPS)
    ]
    y = nc.declare_dram_parameter("y", [P, NT, D], F16, isOutput=True)

    # Pre-clear the semaphore range the tile context will use: runs on
    # gpsimd between the construction barrier and the context entry
    # barrier (idle time), so a re-execution of the NEFF starts from
    # zeroed sems without paying a tail clear + barrier.
    presems = [nc.alloc_semaphore(f"pre{i}") for i in range(32)]
    _SplitDrainTC.precleared = frozenset(s.num for s in presems)
    nc.clear_and_free_semaphores(presems)

    _carriers = []

    with _SplitDrainTC(nc) as tc, ExitStack() as ctx:
        const = ctx.enter_context(tc.tile_pool(name="const", bufs=1))
        xb_pool = ctx.enter_context(tc.tile_pool(name="xb", bufs=1))
        out_pool = ctx.enter_context(tc.tile_pool(name="out", bufs=1))
        # stage and small never recycle: a recycling slot would put a
        # second sync-wait on its writer (the previous reader's engine)
        stage_pool = ctx.enter_context(tc.tile_pool(name="stage", bufs=DIRECT_FROM))
        small = ctx.enter_context(tc.tile_pool(name="small", bufs=NT))
        psum_w = ctx.enter_context(tc.tile_pool(name="psum_w", bufs=1, space="PSUM"))
        psum_o = ctx.enter_context(tc.tile_pool(name="psum_o", bufs=7, space="PSUM"))

        # --- input DMAs: alternate rings, consumption-ordered ----------
        # Tiny SWDGE priming read: the software DGE has a ~3us cold-start
        # on its first transfer; paying it here (gpsimd is idle) makes the
        # mid-run bulk output DMA start promptly.
        prime = const.tile([P, 64], U8)
        nc.gpsimd.dma_start(prime[:], blob[:, 0:64])

        blob_sb = const.tile([P, EBYTES + XTBYTES], U8)
        nc.sync.dma_start(blob_sb[:], blob[:])
        g_sb = []
        for i, (ch, lo, hi) in enumerate(IN_GROUPS):
            sb = xb_pool.tile([P, NCH, (hi - lo) * P], FP8, name=f"g{i}_sb")
            eng = nc.sync if ch == "a" else nc.scalar
            eng.dma_start(sb[:], gparams[i][:])
            g_sb.append(sb)

        # bitcast views of the blob: E_aug [128, 8*129] f16, x tile0 fp8
        rhs16 = blob_sb[:, 0:EBYTES].bitcast(F16)
        x0v = blob_sb[:, EBYTES:EBYTES + XTBYTES].bitcast(FP8)

        # --- PE clock-gate warmup -------------------------------------
        # 1-column memsets just allocate the tiles (the framework rejects
        # read-before-write): the warmup operands are mostly garbage SBUF
        # and the warm PSUM is never read — PE timing is data-independent.
        # Cheap memsets let the first matmul (and the HAM clock ramp it
        # triggers) start ~0.3us earlier than full-tile zeroing.
        wz = const.tile([P, P], FP8)
        ez = const.tile([P, NE], F16)
        nc.vector.memset(wz[:, 0:1], 0.0)
        nc.vector.memset(ez[:, 0:1], 0.0)
        pw = psum_w.tile([P, NE], F32)
        for _ in range(WARMUP_MM):
            nc.tensor.matmul(pw[:], wz[:], ez[:], start=True, stop=True)

        def lhsT(t, c):
            if t == 0:
                return x0v[:, c * P:(c + 1) * P]
            for (ch, lo, hi), sb in zip(IN_GROUPS, g_sb):
                if lo <= t < hi:
                    f = t - lo
                    return sb[:, c, f * P:(f + 1) * P]
            raise AssertionError(t)

        # --- main stream ----------------------------------------------
        out_sb = out_pool.tile([P, NT, D], F16)
        oc = 0
        for t in range(NT):
            po = psum_o.tile([P, NE], F32)
            for c in range(NCH):
                nc.tensor.matmul(po[:], lhsT(t, c), rhs16[:, c * NE:(c + 1) * NE],
                                 start=(c == 0), stop=(c == NCH - 1))
            r = small.tile([P, 1], F32)
            if t < DIRECT_FROM:
                stage = stage_pool.tile([P, NE], F16)
                nc.scalar.copy(stage[:], po[:])
                nc.vector.reciprocal(r[:], stage[:, D:NE])
                nc.vector.tensor_scalar_mul(out_sb[:, t, :], stage[:, 0:D], r[:])
            else:
                # last PSUM rotation: no recycling matmul ever waits on
                # these banks' readers, so DVE can read PSUM directly
                nc.vector.reciprocal(r[:], po[:, D:NE])
                nc.vector.tensor_scalar_mul(out_sb[:, t, :], po[:, 0:D], r[:])
            if oc < len(OUT_CHUNKS) and t == OUT_CHUNKS[oc][2] - 1:
                ch, lo, hi = OUT_CHUNKS[oc]
                eng = {"a": nc.sync, "b": nc.scalar, "s": nc.gpsimd}[ch]
                if ch in ("a", "b"):
                    # 9th+ HWDGE DMA reuses a DMAHW lane; the framework
                    # serializes the reuse with an extra sem wait, which
                    # would exceed the 1-sync-wait cap. Park a carrier nop
                    # right before (same engine, program order) and hoist
                    # the lane wait onto it after sem assignment.
                    carrier = eng.nop(nofuse=True, hint="lane_wait_carrier")
                    dma = eng.dma_start(y[:, lo:hi, :], out_sb[:, lo:hi, :])
                    _carriers.append((carrier, dma))
                else:
                    eng.dma_start(y[:, lo:hi, :], out_sb[:, lo:hi, :])
                oc += 1

    # hoist surplus waits (the DMAHW lane-reuse serialization) from each
    # output DMA onto its carrier nop — waits move earlier on the same
    # engine, which is strictly more conservative
    for carrier, dma in _carriers:
        si = dma.ins.sync_info
        if si is not None and si.on_wait is not None and len(si.on_wait) > 1:
            waits = list(si.on_wait)
            keep = [w for w in waits if "DMAHW" not in (w.ant_name or "")]
            hoist = [w for w in waits if "DMAHW" in (w.ant_name or "")]
            if not keep:
                keep, hoist = waits[:1], waits[1:]
            elif len(keep) > 1:
                hoist += keep[1:]
                keep = keep[:1]
            del si.on_wait[:]
            si.on_wait.extend(keep)
            csi = carrier.ins.sync_info
            if csi is None:
                carrier.ins.sync_info = mybir.SyncInfo(on_update=[],
                                                       on_wait=list(hoist))
            else:
                csi.on_wait.extend(hoist)

    return nc


_cached_nc = None


def make_in_maps(batch_vectors, embedding_matrix):
    """Host-side prep: shard + transpose + swizzle + cast. Layout/dtype only."""
    x = np.asarray(batch_vectors, dtype=np.float32).reshape(B, S, V)
    e = np.asarray(embedding_matrix, dtype=np.float32)
    e_aug = np.empty((V, NE), dtype=np.float16)
    e_aug[:, 0:D] = e.astype(np.float16)
    e_aug[:, D] = np.float16(1.0)
    # [V, NE] -> [p, chunk, NE] -> bytes per partition
    e_dev = np.ascontiguousarray(e_aug.reshape(NCH, P, NE).transpose(1, 0, 2))
    e_bytes = e_dev.reshape(P, EBYTES // 2).view(np.uint8)  # [P, 2064]

    # 0/1 -> fp8 bit pattern, then pure reshape/transpose per tile group
    xb = (x != 0).astype(np.uint8) * np.uint8(FP8_ONE)
    in_maps = []
    for i in range(NCORES):
        shard = xb[i * PER_CORE_B:(i + 1) * PER_CORE_B].reshape(ROWS, V)

        def tiles(lo, hi):
            blk = shard[lo * P:hi * P, :].T          # [V, n*P]
            blk = blk.reshape(NCH, P, (hi - lo) * P).transpose(1, 0, 2)
            return np.ascontiguousarray(blk)

        blob = np.empty((P, EBYTES + XTBYTES), dtype=np.uint8)
        blob[:, 0:EBYTES] = e_bytes
        blob[:, EBYTES:] = tiles(0, 1).reshape(P, XTBYTES)
        m = {"blob": blob}
        for gi, (ch, lo, hi) in enumerate(IN_GROUPS):
            m[f"g{gi}"] = tiles(lo, hi).view(NP_FP8)
        in_maps.append(m)
    return in_maps


def kernel(**inputs):
    global _cached_nc
    from concourse.bass_utils import run_bass_kernel_spmd

    if _cached_nc is None:
        _cached_nc = build_kernel()

    in_maps = make_in_maps(inputs["batch_vectors"], inputs["embedding_matrix"])
    res = run_bass_kernel_spmd(_cached_nc, in_maps, core_ids=list(range(NCORES)))
    out = np.concatenate(
        [
            # y [P, NT, D], row r = t*128 + p  ->  [ROWS, D]
            res.results[i]["y"].astype(np.float32)
            .transpose(1, 0, 2).reshape(PER_CORE_B, S, D)
            for i in range(NCORES)
        ],
        axis=0,
    )
    return out
